# revision 11
# baseline (speedup 1.0000x reference)
"""GCNConv (rank-1 normalized aggregation) Trainium2 kernel, SPMD over 8 cores.

Math (faithful to the torch/jax reference):
    h    = x @ W
    adj  = symmetric 0/1 adjacency from edge_index (duplicates collapse: SET, not add)
    deg  = adj.sum(1);  dinv = 1/sqrt(deg)
    agg  = dinv @ h = (dinv @ x) @ W        # rank-1 identity, [F_OUT]
    out  = dinv[:, None] * agg[None, :] + bias

v5 design (per core; every core reads the full x, output rows are sharded):
  - x scan: 96 TensorE matmuls with the x row-slice [128,128] as the
    STATIONARY operand (bf16 fast weight load) and the dinv column moving:
    v accumulates directly as a [128,1] PSUM column -- no fold/transpose.
  - x chunks front-loaded ([32,16,...,4]) so the last chunk is tiny.
  - tail: cast v to bf16 -> one broadcast agg matmul (agg on all 128
    partitions) -> one cast to bf16 -> each out tile is a single DVE
    tensor_scalar multiply (or ScalarE activation) writing bf16 SBUF
    directly -- no PSUM round-trip, no per-tile matmul+copy.
  - out written bf16 in 3 DMA groups on 2 queues; host upcasts to f32.
  - bias is zero in this workload; a general-bias variant (DVE
    scalar_tensor_tensor with a ones x bias tile) compiles lazily if a
    nonzero bias ever shows up.
  - PE warmed with discarded matmuls in the pre-stream idle window (HAM).

The exact deduplicated degree (an integer/sorting problem, not a flops
problem) is computed on host with np.unique; all O(N*F) floating-point work
runs on the NeuronCores.
"""

import numpy as np

N, F_IN, F_OUT = 12000, 128, 256
N_CORES = 8
ROWS = N // N_CORES            # 1500 output rows per core
NT_OUT = 12                    # 12 row tiles per core (padded)
ROWS_PAD = NT_OUT * 128        # 1536
NT_FULL = 96                   # full-x row slots per partition
N_PAD = NT_FULL * 128          # 12288
# x is streamed as chunks spread over 4 engine HWDGE queues so descriptor
# generation runs in parallel and all 16 DMA engines stay fed. Each entry is
# (engine_name, r-slot count); slots are assigned in order.
# Front-loaded so the final chunk (whose arrival gates the serial tail) is
# tiny; two HWDGE queues stream concurrently over the shared 16-engine pool.
X_PLAN = [
    ("sync", 32), ("scalar", 32),
    ("sync", 16), ("scalar", 8),
    ("sync", 4), ("scalar", 4),
]
N_WARM = 12

_cache = {}


def _build_nc(with_bias: bool):
    import concourse.bacc as bacc
    import concourse.mybir as mybir
    import concourse.tile as tile

    f32 = mybir.dt.float32
    bf16 = mybir.dt.bfloat16

    nc = bacc.Bacc(
        "TRN2",
        target_bir_lowering=False,
        debug=False,
        num_devices=N_CORES,
    )

    x_d = nc.dram_tensor("x", [N_PAD, F_IN], bf16, kind="ExternalInput")
    # cA = [dinvT | W]: dinvT[p, r] = dinv[p*96+r]
    cA_d = nc.dram_tensor("cA", [128, NT_FULL + F_OUT], bf16, kind="ExternalInput")
    # cC[p, n] = dinv[core_row0 + n*128 + p] (per-tile scale columns)
    cC_d = nc.dram_tensor("cC", [128, NT_OUT], f32, kind="ExternalInput")
    if with_bias:
        bias_d = nc.dram_tensor("biasR", [1, F_OUT], bf16, kind="ExternalInput")
    out_d = nc.dram_tensor("out", [ROWS_PAD, F_OUT], bf16, kind="ExternalOutput")

    x_prm = x_d.ap().rearrange("(p r) m -> p r m", p=128)      # [128,96,128]
    out_pnm = out_d.ap().rearrange("(p n) m -> p n m", p=128)  # [128,12,256]

    with tile.TileContext(nc) as tc:
        with (
            tc.tile_pool(name="const", bufs=1) as cpool,
            tc.tile_pool(name="xbuf", bufs=1) as xpool,
            tc.tile_pool(name="obuf", bufs=1) as opool,
            tc.tile_pool(name="pc", bufs=1, space="PSUM") as pcpool,
            tc.tile_pool(name="pa", bufs=1, space="PSUM") as papool,
        ):
            # ---- const DMAs on the scalar queue ----
            cA = cpool.tile([128, NT_FULL + F_OUT], bf16)
            nc.scalar.dma_start(cA[:], cA_d.ap())
            cC = cpool.tile([128, NT_OUT], f32)
            nc.scalar.dma_start(cC[:], cC_d.ap())
            if with_bias:
                biasR = cpool.tile([1, F_OUT], bf16)
                nc.scalar.dma_start(biasR[:], bias_d.ap())

            # ---- x chunk DMAs spread over 4 engine queues (parallel issue,
            # keeps all 16 DMA engines fed; single-queue streaming tops out
            # at ~205 GB/s vs ~330 GB/s aggregate) ----
            engines = {
                "sync": nc.sync,
                "scalar": nc.scalar,
                "gpsimd": nc.gpsimd,
            }
            xc = []
            off = 0
            for q, (ename, sz) in enumerate(X_PLAN):
                t = xpool.tile([128, sz, F_IN], bf16, tag=f"xc{q}", name=f"xc{q}")
                engines[ename].dma_start(t[:], x_prm[:, off : off + sz, :])
                xc.append((t, sz))
                off += sz

            # ---- small SBUF consts (DVE memsets, run early) ----
            wcol = cpool.tile([128, 1], bf16)
            nc.vector.memset(wcol[:], 0.0)
            wrow = cpool.tile([128, F_IN], bf16)
            nc.vector.memset(wrow[:], 0.0)

            pvcol = pcpool.tile([128, 1], f32, tag="pvc", name="pvcol")
            pA2 = papool.tile([128, F_OUT], f32, tag="pA2", name="pA2")

            if with_bias:
                onesrow = cpool.tile([1, 128], bf16)
                nc.vector.memset(onesrow[:], 1.0)
                pB2 = pcpool.tile([128, F_OUT], f32, tag="pB2", name="pB2")
                nc.tensor.matmul(
                    pB2[:], onesrow[:], biasR[:],
                    start=True, stop=True, skip_group_check=True,
                )
                B2 = cpool.tile([128, F_OUT], bf16)
                nc.vector.tensor_copy(B2[:], pB2[:])

            # ---- PE warmup: discarded by the scan's start=True ----
            for i in range(N_WARM):
                nc.tensor.matmul(
                    pvcol[:], wrow[:], wcol[:],
                    start=True, stop=True, skip_group_check=True,
                )

            # ---- x scan: 96 matmuls, x slice stationary, accumulate v col
            rg = 0
            for t, sz in xc:
                for rl in range(sz):
                    nc.tensor.matmul(
                        pvcol[:],
                        t[:, rl, :],
                        cA[:, rg : rg + 1],
                        start=(rg == 0),
                        stop=(rg == NT_FULL - 1),
                        skip_group_check=True,
                    )
                    rg += 1

            # ---- tail: v -> agg broadcast on all partitions -> bf16 ----
            vcol = cpool.tile([128, 1], bf16)
            nc.vector.tensor_copy(vcol[:], pvcol[:])
            nc.tensor.matmul(
                pA2[:], vcol[:].broadcast_to([F_IN, 128]),
                cA[:, NT_FULL : NT_FULL + F_OUT],
                start=True, stop=True, skip_group_check=True,
            )
            A2 = cpool.tile([128, F_OUT], bf16)
            nc.vector.tensor_copy(A2[:], pA2[:])

            # ---- out tiles: 3 groups of 4, each a single DVE tensor_tensor
            # with stride-0 broadcast (A2 along n, cC along m) so 4 tiles cost
            # one instruction instead of 4 serial tensor_scalar ops. ----
            og_tiles = [4, 4, 4]
            og_engines = [nc.sync, nc.scalar, nc.sync]
            base = 0
            for g, gsz in enumerate(og_tiles):
                og = opool.tile([128, gsz, F_OUT], bf16, tag=f"og{g}",
                                name=f"og{g}")
                if with_bias:
                    for j in range(gsz):
                        n = base + j
                        nc.vector.scalar_tensor_tensor(
                            og[:, j, :], A2[:], cC[:, n : n + 1], B2[:],
                            op0=mybir.AluOpType.mult,
                            op1=mybir.AluOpType.add,
                        )
                else:
                    nc.vector.tensor_tensor(
                        og[:],
                        A2[:, None, :].broadcast_to([128, gsz, F_OUT]),
                        cC[:, base : base + gsz, None].broadcast_to(
                            [128, gsz, F_OUT]
                        ),
                        op=mybir.AluOpType.mult,
                    )
                og_engines[g].dma_start(out_pnm[:, base : base + gsz, :], og[:])
                base += gsz

    nc.compile()
    return nc


def _get_nc(with_bias: bool):
    key = f"nc{int(with_bias)}"
    if key not in _cache:
        _cache[key] = _build_nc(with_bias)
    return _cache[key]


def _host_dinv(edge_index: np.ndarray) -> np.ndarray:
    """Exact deduplicated symmetric degree -> 1/sqrt(deg), matching
    adj[a,b]=1; adj[b,a]=1; deg=adj.sum(1)."""
    a = edge_index[0].astype(np.int64)
    b = edge_index[1].astype(np.int64)
    keys = np.unique(np.concatenate([a * N + b, b * N + a]))
    deg = np.bincount(keys // N, minlength=N).astype(np.float32)
    with np.errstate(divide="ignore"):
        dinv = (np.float32(1.0) / np.sqrt(deg)).astype(np.float32)
    return dinv


def kernel(x, edge_index, weight, bias, _trace=False):
    from concourse import bass_utils
    import ml_dtypes

    bf16 = ml_dtypes.bfloat16

    x = np.ascontiguousarray(x, dtype=np.float32)
    weight = np.ascontiguousarray(weight, dtype=np.float32)
    bias = np.ascontiguousarray(bias, dtype=np.float32)
    dinv = _host_dinv(np.asarray(edge_index))

    with_bias = bool(np.any(bias))
    nc = _get_nc(with_bias)

    xp = np.zeros((N_PAD, F_IN), bf16)
    xp[:N] = x.astype(bf16)
    dp = np.zeros((N_PAD,), np.float32)
    dp[:N] = dinv

    cA = np.ascontiguousarray(
        np.concatenate(
            [dp.reshape(128, NT_FULL).astype(bf16), weight.astype(bf16)], axis=1
        )
    )

    in_maps = []
    for c in range(N_CORES):
        r0 = c * ROWS
        ds = np.zeros((ROWS_PAD,), np.float32)
        ds[:ROWS] = dinv[r0 : r0 + ROWS]
        cC = np.ascontiguousarray(ds.reshape(NT_OUT, 128).T)  # [128, 12]
        m = {"x": xp, "cA": cA, "cC": cC}
        if with_bias:
            m["biasR"] = bias.astype(bf16).reshape(1, F_OUT)
        in_maps.append(m)

    res = bass_utils.run_bass_kernel_spmd(
        nc, in_maps, core_ids=list(range(N_CORES)), trace=_trace
    )
    out = np.concatenate(
        [
            np.asarray(res.results[c]["out"])
            .reshape(128, NT_OUT, F_OUT)
            .transpose(1, 0, 2)
            .reshape(ROWS_PAD, F_OUT)[:ROWS]
            for c in range(N_CORES)
        ],
        axis=0,
    ).astype(np.float32)
    if _trace:
        _cache["last_results"] = res
    return out



# revision 17
# speedup vs baseline: 1.0621x; 1.0621x over previous
"""GCNConv (rank-1 normalized aggregation) Trainium2 kernel, SPMD over 8 cores.

Math (faithful to the torch/jax reference):
    h    = x @ W
    adj  = symmetric 0/1 adjacency from edge_index (duplicates collapse: SET, not add)
    deg  = adj.sum(1);  dinv = 1/sqrt(deg)
    agg  = dinv @ h = (dinv @ x) @ W        # rank-1 identity, [F_OUT]
    out  = dinv[:, None] * agg[None, :] + bias

v5 design (per core; every core reads the full x, output rows are sharded):
  - x scan: 96 TensorE matmuls with the x row-slice [128,128] as the
    STATIONARY operand (bf16 fast weight load) and the dinv column moving:
    v accumulates directly as a [128,1] PSUM column -- no fold/transpose.
  - x chunks front-loaded ([32,16,...,4]) so the last chunk is tiny.
  - tail: cast v to bf16 -> one broadcast agg matmul (agg on all 128
    partitions) -> one cast to bf16 -> each out tile is a single DVE
    tensor_scalar multiply (or ScalarE activation) writing bf16 SBUF
    directly -- no PSUM round-trip, no per-tile matmul+copy.
  - out written bf16 in 3 DMA groups on 2 queues; host upcasts to f32.
  - bias is zero in this workload; a general-bias variant (DVE
    scalar_tensor_tensor with a ones x bias tile) compiles lazily if a
    nonzero bias ever shows up.
  - PE warmed with discarded matmuls in the pre-stream idle window (HAM).

The exact deduplicated degree (an integer/sorting problem, not a flops
problem) is computed on host with np.unique; all O(N*F) floating-point work
runs on the NeuronCores.
"""

import numpy as np

N, F_IN, F_OUT = 12000, 128, 256
N_CORES = 8
ROWS = N // N_CORES            # 1500 output rows per core
NT_OUT = 12                    # 12 row tiles per core (padded)
ROWS_PAD = NT_OUT * 128        # 1536
NT_FULL = 96                   # full-x row slots per partition
N_PAD = NT_FULL * 128          # 12288
# x is streamed as chunks spread over 4 engine HWDGE queues so descriptor
# generation runs in parallel and all 16 DMA engines stay fed. Each entry is
# (engine_name, r-slot count); slots are assigned in order.
# Front-loaded so the final chunk (whose arrival gates the serial tail) is
# tiny; two HWDGE queues stream concurrently over the shared 16-engine pool.
X_PLAN = [
    ("scalar", 44), ("sync", 44),
    ("scalar", 4), ("sync", 4),
]
N_WARM = 12

_cache = {}


def _build_nc(with_bias: bool):
    import concourse.bacc as bacc
    import concourse.mybir as mybir
    import concourse.tile as tile

    f32 = mybir.dt.float32
    bf16 = mybir.dt.bfloat16

    nc = bacc.Bacc(
        "TRN2",
        target_bir_lowering=False,
        debug=False,
        num_devices=N_CORES,
    )

    x_d = nc.dram_tensor("x", [N_PAD, F_IN], bf16, kind="ExternalInput")
    # cA = [dinvT | W]: dinvT[p, r] = dinv[p*96+r]
    cA_d = nc.dram_tensor("cA", [128, NT_FULL + F_OUT], bf16, kind="ExternalInput")
    # cC[p, n] = dinv[core_row0 + n*128 + p] (per-tile scale columns)
    cC_d = nc.dram_tensor("cC", [128, NT_OUT], f32, kind="ExternalInput")
    if with_bias:
        bias_d = nc.dram_tensor("biasR", [1, F_OUT], bf16, kind="ExternalInput")
    out_d = nc.dram_tensor("out", [ROWS_PAD, F_OUT], bf16, kind="ExternalOutput")

    x_prm = x_d.ap().rearrange("(p r) m -> p r m", p=128)      # [128,96,128]
    out_pnm = out_d.ap().rearrange("(p n) m -> p n m", p=128)  # [128,12,256]

    with tile.TileContext(nc) as tc:
        with (
            tc.tile_pool(name="const", bufs=1) as cpool,
            tc.tile_pool(name="xbuf", bufs=1) as xpool,
            tc.tile_pool(name="obuf", bufs=1) as opool,
            tc.tile_pool(name="pc", bufs=1, space="PSUM") as pcpool,
            tc.tile_pool(name="pa", bufs=1, space="PSUM") as papool,
        ):
            # ---- const DMAs on the sync queue (scalar leads with x) ----
            cA = cpool.tile([128, NT_FULL + F_OUT], bf16)
            nc.sync.dma_start(cA[:], cA_d.ap())
            cC = cpool.tile([128, NT_OUT], f32)
            nc.sync.dma_start(cC[:], cC_d.ap())
            if with_bias:
                biasR = cpool.tile([1, F_OUT], bf16)
                nc.scalar.dma_start(biasR[:], bias_d.ap())

            # ---- x chunk DMAs spread over 4 engine queues (parallel issue,
            # keeps all 16 DMA engines fed; single-queue streaming tops out
            # at ~205 GB/s vs ~330 GB/s aggregate) ----
            engines = {
                "sync": nc.sync,
                "scalar": nc.scalar,
                "gpsimd": nc.gpsimd,
            }
            xc = []
            off = 0
            for q, (ename, sz) in enumerate(X_PLAN):
                t = xpool.tile([128, sz, F_IN], bf16, tag=f"xc{q}", name=f"xc{q}")
                engines[ename].dma_start(t[:], x_prm[:, off : off + sz, :])
                xc.append((t, sz))
                off += sz

            # ---- small SBUF consts (DVE memsets, run early) ----
            wcol = cpool.tile([128, 1], bf16)
            nc.vector.memset(wcol[:], 0.0)
            wrow = cpool.tile([128, F_IN], bf16)
            nc.vector.memset(wrow[:], 0.0)

            pvcol = pcpool.tile([128, 1], f32, tag="pvc", name="pvcol")
            pA2 = papool.tile([128, F_OUT], f32, tag="pA2", name="pA2")

            if with_bias:
                onesrow = cpool.tile([1, 128], bf16)
                nc.vector.memset(onesrow[:], 1.0)
                pB2 = pcpool.tile([128, F_OUT], f32, tag="pB2", name="pB2")
                nc.tensor.matmul(
                    pB2[:], onesrow[:], biasR[:],
                    start=True, stop=True, skip_group_check=True,
                )
                B2 = cpool.tile([128, F_OUT], bf16)
                nc.vector.tensor_copy(B2[:], pB2[:])

            # ---- PE warmup: discarded by the scan's start=True ----
            for i in range(N_WARM):
                nc.tensor.matmul(
                    pvcol[:], wrow[:], wcol[:],
                    start=True, stop=True, skip_group_check=True,
                )

            # ---- x scan: 96 matmuls, x slice stationary, accumulate v col
            rg = 0
            for t, sz in xc:
                for rl in range(sz):
                    nc.tensor.matmul(
                        pvcol[:],
                        t[:, rl, :],
                        cA[:, rg : rg + 1],
                        start=(rg == 0),
                        stop=(rg == NT_FULL - 1),
                        skip_group_check=True,
                    )
                    rg += 1

            # ---- tail: v -> agg broadcast on all partitions -> bf16 ----
            vcol = cpool.tile([128, 1], bf16)
            nc.vector.tensor_copy(vcol[:], pvcol[:])
            nc.tensor.matmul(
                pA2[:], vcol[:].broadcast_to([F_IN, 128]),
                cA[:, NT_FULL : NT_FULL + F_OUT],
                start=True, stop=True, skip_group_check=True,
            )
            A2 = cpool.tile([128, F_OUT], bf16)
            nc.vector.tensor_copy(A2[:], pA2[:])

            # ---- out tiles: 3 groups of 4. All-bf16 tensor_scalar keeps DVE
            # on its fast 16-bit path (~130ns/tile); ScalarE takes one tile
            # per group in parallel. ----
            og_tiles = [4, 4, 4]
            og_engines = [nc.sync, nc.scalar, nc.sync]
            scalar_tiles = {0, 4, 8}
            base = 0
            for g, gsz in enumerate(og_tiles):
                og = opool.tile([128, gsz, F_OUT], bf16, tag=f"og{g}",
                                name=f"og{g}")
                for j in range(gsz):
                    n = base + j
                    dst = og[:, j, :]
                    if with_bias:
                        nc.vector.scalar_tensor_tensor(
                            dst, A2[:], cC[:, n : n + 1], B2[:],
                            op0=mybir.AluOpType.mult,
                            op1=mybir.AluOpType.add,
                        )
                    elif n in scalar_tiles:
                        nc.scalar.activation(
                            dst, A2[:], mybir.ActivationFunctionType.Copy,
                            scale=cC[:, n : n + 1],
                        )
                    else:
                        nc.vector.tensor_scalar_mul(dst, A2[:], cC[:, n : n + 1])
                og_engines[g].dma_start(out_pnm[:, base : base + gsz, :], og[:])
                base += gsz

    nc.compile()
    return nc


def _get_nc(with_bias: bool):
    key = f"nc{int(with_bias)}"
    if key not in _cache:
        _cache[key] = _build_nc(with_bias)
    return _cache[key]


def _host_dinv(edge_index: np.ndarray) -> np.ndarray:
    """Exact deduplicated symmetric degree -> 1/sqrt(deg), matching
    adj[a,b]=1; adj[b,a]=1; deg=adj.sum(1)."""
    a = edge_index[0].astype(np.int64)
    b = edge_index[1].astype(np.int64)
    keys = np.unique(np.concatenate([a * N + b, b * N + a]))
    deg = np.bincount(keys // N, minlength=N).astype(np.float32)
    with np.errstate(divide="ignore"):
        dinv = (np.float32(1.0) / np.sqrt(deg)).astype(np.float32)
    return dinv


def kernel(x, edge_index, weight, bias, _trace=False):
    from concourse import bass_utils
    import ml_dtypes

    bf16 = ml_dtypes.bfloat16

    x = np.ascontiguousarray(x, dtype=np.float32)
    weight = np.ascontiguousarray(weight, dtype=np.float32)
    bias = np.ascontiguousarray(bias, dtype=np.float32)
    dinv = _host_dinv(np.asarray(edge_index))

    with_bias = bool(np.any(bias))
    nc = _get_nc(with_bias)

    xp = np.zeros((N_PAD, F_IN), bf16)
    xp[:N] = x.astype(bf16)
    dp = np.zeros((N_PAD,), np.float32)
    dp[:N] = dinv

    cA = np.ascontiguousarray(
        np.concatenate(
            [dp.reshape(128, NT_FULL).astype(bf16), weight.astype(bf16)], axis=1
        )
    )

    in_maps = []
    for c in range(N_CORES):
        r0 = c * ROWS
        ds = np.zeros((ROWS_PAD,), np.float32)
        ds[:ROWS] = dinv[r0 : r0 + ROWS]
        cC = np.ascontiguousarray(ds.reshape(NT_OUT, 128).T)
        m = {"x": xp, "cA": cA, "cC": cC}
        if with_bias:
            m["biasR"] = bias.astype(bf16).reshape(1, F_OUT)
        in_maps.append(m)

    res = bass_utils.run_bass_kernel_spmd(
        nc, in_maps, core_ids=list(range(N_CORES)), trace=_trace
    )
    out = np.concatenate(
        [
            np.asarray(res.results[c]["out"])
            .reshape(128, NT_OUT, F_OUT)
            .transpose(1, 0, 2)
            .reshape(ROWS_PAD, F_OUT)[:ROWS]
            for c in range(N_CORES)
        ],
        axis=0,
    ).astype(np.float32)
    if _trace:
        _cache["last_results"] = res
    return out

